# revision 11
# baseline (speedup 1.0000x reference)
"""Dual-stream attention kernel for Trainium2 (8 NeuronCores, SPMD).

Problem: B=4, S=4096, DIM=256
  out1 = LN(mean(x1,1) + softmax(mask(sum_j tanh(k1 @ q2.T))) @ v1)
  out2 = LN(mean(x2,1) + softmax(mask(sum_j tanh(k2 @ q1.T))) @ v2)

Sharding: 8 independent (batch, stream) units -> one per core, no
cross-core communication. Core 2*b+s handles batch b, stream s.
"""

import numpy as np

B, S, DIM = 4, 4096, 256
P = 128
MB = DIM // P      # 2 d-blocks of 128 partitions
SB = S // P        # 32 seq blocks of 128
JC = 512           # score j-chunk (one PSUM bank of fp32)
NJ = S // JC       # 8 chunks
EPS = 1e-5
NCORES = 8

_PROG = None       # cached Bass program (compiled once per process)


def _build_program():
    import concourse.bacc as bacc
    import concourse.tile as tile
    from concourse import mybir
    from concourse.masks import make_identity

    f32 = mybir.dt.float32
    f32r = mybir.dt.float32r
    AF = mybir.ActivationFunctionType
    AX = mybir.AxisListType
    OP = mybir.AluOpType

    nc = bacc.Bacc("TRN2", target_bir_lowering=False, debug=False)

    # ---- DRAM I/O (per-core data; weights replicated) ----
    xaT_d = nc.declare_dram_parameter("xaT", [DIM, S], f32r, False)
    xbT_d = nc.declare_dram_parameter("xbT", [DIM, S], f32r, False)
    wkT_d = nc.declare_dram_parameter("wkT", [DIM, DIM], f32r, False)
    wqT_d = nc.declare_dram_parameter("wqT", [DIM, DIM], f32r, False)
    wvT_d = nc.declare_dram_parameter("wvT", [DIM, DIM], f32r, False)
    bk_d = nc.declare_dram_parameter("bk", [P, MB], f32, False)
    bq_d = nc.declare_dram_parameter("bq", [P, MB], f32, False)
    bv_d = nc.declare_dram_parameter("bv", [1, DIM], f32r, False)
    madd_d = nc.declare_dram_parameter("madd", [P, SB], f32, False)
    gamma_d = nc.declare_dram_parameter("gamma", [P, MB], f32, False)
    beta_d = nc.declare_dram_parameter("beta", [P, MB], f32, False)
    sel_d = nc.declare_dram_parameter("sel", [4, 2], f32, False)
    onesr_d = nc.declare_dram_parameter("onesr", [1, P], f32r, False)
    out_d = nc.declare_dram_parameter("out", [P, MB], f32, True)

    with tile.TileContext(nc) as tc:
        with (
            tc.tile_pool(name="const", bufs=1) as const,
            tc.tile_pool(name="big", bufs=1) as big,
            tc.tile_pool(name="work", bufs=2) as work,
            tc.tile_pool(name="csums", bufs=4) as csums_pool,
            tc.tile_pool(name="mmps", bufs=4, space="PSUM") as mm_psum,
            tc.tile_pool(name="vpps", bufs=2, space="PSUM") as vp_psum,
            tc.tile_pool(name="vecps", bufs=1, space="PSUM") as vec_psum,
        ):
            # ---- load constants/weights ----
            wk = [const.tile([P, DIM], f32r, tag=f"wk{k}", name=f"wk{k}") for k in range(MB)]
            wq = [const.tile([P, DIM], f32r, tag=f"wq{k}", name=f"wq{k}") for k in range(MB)]
            wv = [const.tile([P, DIM], f32r, tag=f"wv{k}", name=f"wv{k}") for k in range(MB)]
            for k in range(MB):
                nc.sync.dma_start(out=wk[k], in_=wkT_d[k * P:(k + 1) * P, :])
                nc.sync.dma_start(out=wq[k], in_=wqT_d[k * P:(k + 1) * P, :])
                nc.sync.dma_start(out=wv[k], in_=wvT_d[k * P:(k + 1) * P, :])
            bk_sb = const.tile([P, MB], f32, tag="bk")
            bq_sb = const.tile([P, MB], f32, tag="bq")
            bv_sb = const.tile([1, DIM], f32r, tag="bv")
            madd_sb = const.tile([P, SB], f32, tag="madd")
            gamma_sb = const.tile([P, MB], f32, tag="gamma")
            beta_sb = const.tile([P, MB], f32, tag="beta")
            nc.sync.dma_start(out=bk_sb, in_=bk_d[:, :])
            nc.sync.dma_start(out=bq_sb, in_=bq_d[:, :])
            nc.sync.dma_start(out=bv_sb, in_=bv_d[:, :])
            nc.sync.dma_start(out=madd_sb, in_=madd_d[:, :])
            nc.sync.dma_start(out=gamma_sb, in_=gamma_d[:, :])
            nc.sync.dma_start(out=beta_sb, in_=beta_d[:, :])
            ones_row = const.tile([1, P], f32, tag="ones")
            nc.gpsimd.memset(ones_row, 1.0)
            ones_row_r = const.tile([1, P], f32r, tag="onesr")
            nc.sync.dma_start(out=ones_row_r, in_=onesr_d[:, :])
            ones_col = const.tile([P, 1], f32, tag="onesc")
            nc.gpsimd.memset(ones_col, 1.0)
            eps_sb = const.tile([P, 1], f32, tag="eps")
            nc.gpsimd.memset(eps_sb, EPS)
            sel_sb = const.tile([4, 2], f32, tag="sel")
            nc.sync.dma_start(out=sel_sb, in_=sel_d[:, :])
            ident = const.tile([P, P], f32, tag="ident")
            make_identity(nc, ident)

            # ---- load activations (transposed layout [d, s]) ----
            xa = [big.tile([P, S], f32r, tag=f"xa{k}", name=f"xa{k}") for k in range(MB)]
            xb = [big.tile([P, S], f32r, tag=f"xb{k}", name=f"xb{k}") for k in range(MB)]
            for k in range(MB):
                nc.sync.dma_start(out=xa[k], in_=xaT_d[k * P:(k + 1) * P, :])
            for k in range(MB):
                nc.sync.dma_start(out=xb[k], in_=xbT_d[k * P:(k + 1) * P, :])

            # ---- row-sum of xa (for mean over seq) ----
            xsum = work.tile([P, MB], f32, tag="xsum")
            for k in range(MB):
                nc.vector.reduce_sum(out=xsum[:, k:k + 1], in_=xa[k], axis=AX.X)

            # ---- projections k,q in [d, s] layout (relu+bias on DVE) ----
            kt = [big.tile([P, S], f32r, tag=f"kt{k}", name=f"kt{k}") for k in range(MB)]
            qt = [big.tile([P, S], f32r, tag=f"qt{k}", name=f"qt{k}") for k in range(MB)]
            for dst, wsb, bsb, src in ((kt, wk, bk_sb, xa), (qt, wq, bq_sb, xb)):
                for m in range(MB):
                    for ng in range(2):  # groups of 4 n-chunks: weight reuse
                        pss = [mm_psum.tile([P, JC], f32, tag="mm", name=f"pss{j}")
                               for j in range(4)]
                        for kk in range(MB):
                            for j in range(4):
                                n = ng * 4 + j
                                nc.tensor.matmul(
                                    pss[j],
                                    lhsT=wsb[kk][:, m * P:(m + 1) * P],
                                    rhs=src[kk][:, n * JC:(n + 1) * JC],
                                    start=(kk == 0), stop=(kk == MB - 1),
                                )
                        for j in range(4):
                            n = ng * 4 + j
                            # out = max(psum + bias, 0)
                            nc.vector.tensor_scalar(
                                out=dst[m][:, n * JC:(n + 1) * JC],
                                in0=pss[j],
                                scalar1=bsb[:, m:m + 1], scalar2=0.0,
                                op0=OP.add, op1=OP.max,
                            )

            # ---- projection v in natural layout [s, d] ----
            v_sb = big.tile([P, SB, DIM], f32, tag="v")
            for si in range(SB):
                ps = vp_psum.tile([P, DIM], f32, tag="vp")
                for kk in range(MB):
                    nc.tensor.matmul(
                        ps,
                        lhsT=xa[kk][:, si * P:(si + 1) * P],
                        rhs=wv[kk],
                        start=(kk == 0), stop=False,
                    )
                # += 1 x bv  (broadcast bias along partitions)
                nc.tensor.matmul(
                    ps, lhsT=ones_row_r, rhs=bv_sb,
                    start=False, stop=True,
                )
                nc.vector.tensor_scalar_max(
                    out=v_sb[:, si, :], in0=ps, scalar1=0.0)

            # ---- scores: s[i] = sum_j tanh(k[i] . q[j]) ----
            s_sb = work.tile([P, SB], f32, tag="s")
            for ib in range(SB):
                cs = csums_pool.tile([P, NJ], f32, tag="cs")
                for jg in range(2):
                    pss = [mm_psum.tile([P, JC], f32, tag="mm", name=f"pss{j}")
                           for j in range(4)]
                    for kk in range(MB):
                        for j in range(4):
                            n = jg * 4 + j
                            nc.tensor.matmul(
                                pss[j],
                                lhsT=kt[kk][:, ib * P:(ib + 1) * P],
                                rhs=qt[kk][:, n * JC:(n + 1) * JC],
                                start=(kk == 0), stop=(kk == MB - 1),
                            )
                    for j in range(4):
                        n = jg * 4 + j
                        # tanh in place in PSUM; row-sum via accum_out
                        nc.scalar.activation(
                            out=pss[j], in_=pss[j], func=AF.Tanh,
                            accum_out=cs[:, n:n + 1],
                        )
                nc.vector.reduce_sum(out=s_sb[:, ib:ib + 1], in_=cs, axis=AX.X)

            # ---- masked softmax over all 4096 scores ----
            # cross-partition reductions via PE (transpose / ones-matmuls)
            m_sb = work.tile([P, SB], f32, tag="msk")
            nc.vector.tensor_add(m_sb, s_sb, madd_sb)
            mp = work.tile([P, 1], f32, tag="mp")
            nc.vector.reduce_max(out=mp, in_=m_sb, axis=AX.X)
            mpT_ps = vec_psum.tile([1, P], f32, tag="tiny", name="mpT_ps")
            nc.tensor.transpose(out=mpT_ps, in_=mp, identity=ident)
            mpT_sb = work.tile([1, P], f32, tag="mpT")
            nc.vector.tensor_copy(out=mpT_sb, in_=mpT_ps)
            mx1 = work.tile([1, 1], f32, tag="mx1")
            nc.vector.reduce_max(out=mx1, in_=mpT_sb, axis=AX.X)
            mneg1 = work.tile([1, 1], f32, tag="mneg1")
            nc.vector.tensor_scalar_mul(out=mneg1, in0=mx1, scalar1=-1.0)
            negm_ps = vec_psum.tile([P, 1], f32, tag="tiny", name="negm_ps")
            nc.tensor.matmul(negm_ps, lhsT=ones_row,
                             rhs=mneg1, start=True, stop=True)
            negm = work.tile([P, 1], f32, tag="negm")
            nc.vector.tensor_copy(out=negm, in_=negm_ps)
            e_sb = work.tile([P, SB], f32, tag="e")
            zp = work.tile([P, 1], f32, tag="zp")
            nc.scalar.activation(
                out=e_sb, in_=m_sb, func=AF.Exp, bias=negm, accum_out=zp)
            z_ps = vec_psum.tile([1, 1], f32, tag="tiny", name="z_ps")
            nc.tensor.matmul(z_ps, lhsT=zp,
                             rhs=ones_col, start=True, stop=True)
            z_sb = work.tile([1, 1], f32, tag="z1")
            nc.vector.tensor_copy(out=z_sb, in_=z_ps)
            invz1 = work.tile([1, 1], f32, tag="invz1")
            nc.vector.reciprocal(out=invz1, in_=z_sb)
            invz_ps = vec_psum.tile([P, 1], f32, tag="tiny", name="invz_ps")
            nc.tensor.matmul(invz_ps, lhsT=ones_row,
                             rhs=invz1, start=True, stop=True)
            invz = work.tile([P, 1], f32, tag="invz")
            nc.vector.tensor_copy(out=invz, in_=invz_ps)

            # ---- vec = sum_s e[s] * v[s, :]  (then scale by 1/Z) ----
            vecp = vec_psum.tile([P, MB], f32, tag="vec")
            for m in range(MB):
                for si in range(SB):
                    nc.tensor.matmul(
                        vecp[:, m:m + 1],
                        lhsT=v_sb[:, si, m * P:(m + 1) * P],
                        rhs=e_sb[:, si:si + 1],
                        start=(si == 0), stop=(si == SB - 1),
                    )

            # ---- y = mean(xa) + vec/Z ----
            vscaled = work.tile([P, MB], f32, tag="vs")
            nc.vector.tensor_scalar_mul(out=vscaled, in0=vecp, scalar1=invz)
            # stat4 cols: [y0, y1, y0^2, y1^2]; y_sb aliases cols 0:2
            stat4 = work.tile([P, 4], f32, tag="stat4")
            y_sb = stat4[:, 0:MB]
            nc.vector.scalar_tensor_tensor(
                out=y_sb, in0=xsum, scalar=1.0 / S, in1=vscaled,
                op0=OP.mult, op1=OP.add)

            # ---- layernorm over d=256 (spans 2 partition blocks) ----
            nc.vector.tensor_mul(stat4[:, MB:2 * MB], y_sb, y_sb)
            r4_ps = vec_psum.tile([4, 1], f32, tag="tiny", name="r4_ps")
            nc.tensor.matmul(r4_ps, lhsT=stat4,
                             rhs=ones_col, start=True, stop=True)
            r4 = work.tile([4, 1], f32, tag="r4")
            nc.vector.tensor_copy(out=r4, in_=r4_ps)
            s12_ps = vec_psum.tile([1, 2], f32, tag="tiny", name="s12_ps")
            nc.tensor.matmul(s12_ps, lhsT=r4,
                             rhs=sel_sb, start=True, stop=True)
            s12 = work.tile([1, 2], f32, tag="s12")
            nc.vector.tensor_copy(out=s12, in_=s12_ps)
            # mu = sum(y)/D ; ex2 = sum(y^2)/D ; var = ex2 - mu^2
            mu1 = work.tile([1, 1], f32, tag="mu1")
            nc.vector.tensor_scalar_mul(out=mu1, in0=s12[:, 0:1],
                                        scalar1=1.0 / DIM)
            ex2 = work.tile([1, 1], f32, tag="ex2")
            nc.vector.tensor_scalar_mul(out=ex2, in0=s12[:, 1:2],
                                        scalar1=1.0 / DIM)
            mu2 = work.tile([1, 1], f32, tag="mu2")
            nc.vector.tensor_mul(mu2, mu1, mu1)
            var = work.tile([1, 1], f32, tag="var")
            nc.vector.tensor_sub(var, ex2, mu2)
            # rstd = exp(-0.5*ln(var+eps))  (ln/exp share a table set)
            lnv = work.tile([1, 1], f32, tag="lnv")
            nc.scalar.activation(out=lnv, in_=var, func=AF.Ln,
                                 bias=eps_sb[0:1, :])
            rstd1 = work.tile([1, 1], f32, tag="rstd1")
            nc.scalar.activation(out=rstd1, in_=lnv, func=AF.Exp, scale=-0.5)
            # broadcast [mu, rstd] to all partitions
            mr1 = work.tile([1, 2], f32, tag="mr1")
            nc.vector.tensor_copy(out=mr1[:, 0:1], in_=mu1)
            nc.vector.tensor_copy(out=mr1[:, 1:2], in_=rstd1)
            mr_ps = vec_psum.tile([P, 2], f32, tag="tiny", name="mr_ps")
            nc.tensor.matmul(mr_ps, lhsT=ones_row,
                             rhs=mr1, start=True, stop=True)
            mr_sb = work.tile([P, 2], f32, tag="mr")
            nc.vector.tensor_copy(out=mr_sb, in_=mr_ps)
            # (y - mu) * rstd
            norm = work.tile([P, MB], f32, tag="norm")
            nc.vector.tensor_scalar(
                out=norm, in0=y_sb, scalar1=mr_sb[:, 0:1],
                scalar2=mr_sb[:, 1:2], op0=OP.subtract, op1=OP.mult)
            normg = work.tile([P, MB], f32, tag="normg")
            nc.vector.tensor_mul(normg, norm, gamma_sb)
            out_sb = work.tile([P, MB], f32, tag="out")
            nc.vector.tensor_add(out_sb, normg, beta_sb)
            nc.sync.dma_start(out=out_d[:, :], in_=out_sb)

    nc.finalize()
    return nc


def _get_program():
    global _PROG
    if _PROG is None:
        _PROG = _build_program()
    return _PROG


def _pn(v):
    """[DIM] -> [P, MB] with tile[p, m] = v[m*128 + p]."""
    return np.ascontiguousarray(np.asarray(v, np.float32).reshape(MB, P).T)


def make_in_maps(fingerprint_vectors1, fingerprint_vectors2, mask1, mask2,
                 Wq, bq, Wk, bk, Wv, bv, gamma, beta):
    x1 = np.asarray(fingerprint_vectors1, np.float32)
    x2 = np.asarray(fingerprint_vectors2, np.float32)
    m1 = np.asarray(mask1, bool)
    m2 = np.asarray(mask2, bool)
    x1T = np.ascontiguousarray(x1.transpose(0, 2, 1))  # [B, D, S]
    x2T = np.ascontiguousarray(x2.transpose(0, 2, 1))
    wqT = np.ascontiguousarray(np.asarray(Wq, np.float32).T)
    wkT = np.ascontiguousarray(np.asarray(Wk, np.float32).T)
    wvT = np.ascontiguousarray(np.asarray(Wv, np.float32).T)
    shared = {
        "wkT": wkT, "wqT": wqT, "wvT": wvT,
        "bk": _pn(bk), "bq": _pn(bq),
        "bv": np.ascontiguousarray(np.asarray(bv, np.float32).reshape(1, DIM)),
        "gamma": _pn(gamma), "beta": _pn(beta),
        "sel": np.array([[1, 0], [1, 0], [0, 1], [0, 1]], np.float32),
        "onesr": np.ones((1, P), np.float32),
    }
    in_maps = []
    for b in range(B):
        for stream in range(2):
            if stream == 0:
                xa, xbt, msk = x1T[b], x2T[b], m1[b]
            else:
                xa, xbt, msk = x2T[b], x1T[b], m2[b]
            madd = np.where(msk, np.float32(-1e30), np.float32(0.0))
            madd = np.ascontiguousarray(
                madd.astype(np.float32).reshape(SB, P).T)
            in_maps.append(dict(shared, xaT=xa, xbT=xbt, madd=madd))
    return in_maps


# test.py can flip these to get a profile out of the run
RUN_OPTS = {"trace": False, "trace_kwargs": None}
LAST = {}


def kernel(**inputs):
    from concourse.bass_utils import run_bass_kernel_spmd

    nc = _get_program()
    in_maps = make_in_maps(**inputs)
    kw = {}
    if RUN_OPTS.get("trace"):
        kw["trace"] = True
        if RUN_OPTS.get("trace_kwargs"):
            kw["trace_kwargs"] = RUN_OPTS["trace_kwargs"]
    res = run_bass_kernel_spmd(nc, in_maps, list(range(NCORES)), **kw)
    LAST["exec_time_ns"] = res.exec_time_ns
    LAST["profile_json"] = res.profile_json
    outs = res.results
    out1 = np.stack([np.asarray(outs[2 * b]["out"]).T.reshape(DIM)
                     for b in range(B)])
    out2 = np.stack([np.asarray(outs[2 * b + 1]["out"]).T.reshape(DIM)
                     for b in range(B)])
    return out1.astype(np.float32), out2.astype(np.float32)
